# revision 52
# baseline (speedup 1.0000x reference)
"""GraphSAGE (gnn_message_passing) forward pass on 8 Trainium2 NeuronCores.

Sharding (hardcoded): row-shard the 10000 nodes across 8 cores (1250 each,
padded to 1280).  The row-normalized adjacency shard is staged host-side as
fp8e4m3 ([10240, 1280] transposed, scaled by 4096 with the inverse scale
folded into W_neigh) and loaded into SBUF once -- both GNN layers aggregate
from the same resident/streamed copy.  Node features travel between layers
via fp8 AllGathers (two halves each, pipelined against the aggregation
matmuls).  Small weights / LSTM params are replicated.

The LSTM is computed in "tanh-only" form (sigmoid(x) = 0.5*tanh(x/2)+0.5,
with the 0.5 factors folded into Whh/Wih1/biases and cell/h states kept
doubled) so the scalar engine never swaps activation tables inside the
recurrence; elu/softmax stages are likewise grouped by activation function
(activation-table loads cost ~1.3us each).
"""

import os
from contextlib import ExitStack

import numpy as np
import ml_dtypes

import concourse.bass as bass
import concourse.bacc as bacc
import concourse.mybir as mybir
import concourse.tile as tile
from concourse.bass_utils import run_bass_kernel_spmd
from concourse.masks import make_identity

F32 = mybir.dt.float32
BF16 = mybir.dt.bfloat16
FP8 = mybir.dt.float8e4
AX = mybir.AxisListType
OP = mybir.AluOpType
AF = mybir.ActivationFunctionType

# ---- problem constants (hardcoded per spec) ----
N = 10000        # nodes
NC = 8           # cores
NPC = 1250       # original nodes per core
PC = 1280        # padded nodes per core
NP = NC * PC     # padded total nodes = 10240
KT = NP // 128   # 80 contraction tiles
IT = PC // 128   # 10 node tiles per core
HT = 5           # k-tiles per gather half per core
NFEAT = 2000
FPAD = 2048
FT = FPAD // 128  # 16
NH = 128
NHE = 64
NFE = 256
D = NH + NHE     # 192
NOUT = 20
L = 2
BN_EPS = 1e-5
ADJ_SCALE = 4096.0
NRES = 4         # adjacency chunks resident in SBUF across both layers
NE = 8           # x eighths
EW = PC // NE    # 160

CHUNKS = [(0, 512), (512, 512), (1024, 256)]

LAST_RESULT = None  # test.py reads exec_time info from here

_CACHED_NC = None


def _bf(a):
    return np.asarray(a, dtype=ml_dtypes.bfloat16)


def _f32(a):
    return np.ascontiguousarray(a, dtype=np.float32)


# --------------------------------------------------------------------------
# device program
# --------------------------------------------------------------------------

def _build_program():
    nc = bacc.Bacc("TRN2", target_bir_lowering=False, debug=False, num_devices=NC)

    def inp(name, shape, dtype):
        return nc.declare_dram_parameter(name, list(shape), dtype, isOutput=False)

    # per-core tensors
    d_adj = inp("adj8", [2, NC, 128, HT, PC], FP8)   # [half, chunk, p, s, i]
    d_x = inp("x8", [NE, 128, FT, EW], BF16)
    d_emb = inp("embT", [128, 2, PC], BF16)
    # replicated weights
    d_w_inT = inp("w_inT", [FPAD, NH], BF16)
    d_wgs_sT = inp("wgs_sT", [L, NH, NH], BF16)
    d_wgs_nT = inp("wgs_nT", [L, NH, NH], BF16)      # pre-scaled by 1/ADJ_SCALE
    d_bgs = inp("bgs", [NH, L], F32)
    d_wihT = inp("wihT", [L, NH, 4 * NH], BF16)      # gate scales pre-folded
    d_whhT = inp("whhT", [L, NH, 4 * NH], BF16)      # gate scales pre-folded
    d_blstm = inp("blstm", [NH, 2 * 4], F32)         # i/f/o pre-scaled by 0.5
    d_blr = inp("blr", [1, 8 * NH], BF16)            # same, as row vector
    d_w_embT = inp("w_embT", [NFE, NHE], BF16)
    d_w_fcT = inp("w_fcT", [D, D], BF16)
    d_w_outT = inp("w_outT", [D, NOUT], BF16)
    d_bout = inp("bout_col", [NOUT, 1], F32)
    d_sm = {}
    for nm, p in [("sc_in", NH), ("sh_in", NH), ("sc_in_h", NH), ("sh_in2", NH),
                  ("sc_emb", NHE), ("sh_emb", NHE),
                  ("sc_fc_a", 128), ("sh_fc_a", 128),
                  ("sc_fc_b", 64), ("sh_fc_b", 64)]:
        d_sm[nm] = inp(nm, [p, 1], F32)
    d_out = nc.declare_dram_parameter("out", [128, IT * NOUT], F32, isOutput=True)

    # internal DRAM for collectives: both layers gather in halves; the
    # second half's AllGather chains on the CC stream and hides under the
    # first half's aggregation matmuls.
    bounce = [[nc.dram_tensor(f"bounce{l}_{h}", [128, HT * 128], FP8)
               for h in range(2)] for l in range(L)]
    hg = [[nc.dram_tensor(f"hg{l}_{h}", [NC, 128, HT * 128], FP8,
                          addr_space="Shared") for h in range(2)]
          for l in range(L)]
    # tiny pre-sync AllGathers: absorb cross-core skew on the CC stream
    # just before each real gather batch, so the real ones chain at data
    # speed instead of paying the straggler wait themselves
    dmyb = [nc.dram_tensor(f"dmyb{i}", [16, 128], BF16) for i in range(2)]
    dmyg = [nc.dram_tensor(f"dmyg{i}", [NC, 16, 128], BF16,
                           addr_space="Shared") for i in range(2)]
    groups = [list(range(NC))]

    with tile.TileContext(nc) as tc, ExitStack() as top:
        const = top.enter_context(tc.tile_pool(name="const", bufs=1))
        persist = top.enter_context(tc.tile_pool(name="persist", bufs=1))
        padjr = top.enter_context(tc.tile_pool(name="adjr", bufs=1))
        padjs = top.enter_context(tc.tile_pool(name="adjs", bufs=3))
        px = top.enter_context(tc.tile_pool(name="px", bufs=2))
        pnat = top.enter_context(tc.tile_pool(name="pnat", bufs=2))
        ploc = top.enter_context(tc.tile_pool(name="ploc", bufs=2))
        ptmp = top.enter_context(tc.tile_pool(name="ptmp", bufs=1))
        pg = top.enter_context(tc.tile_pool(name="pg", bufs=1))

        # ---- tiles for constants (DMAs issued in ring-order below) ----
        w_in = const.tile([128, FPAD], BF16, tag="w_in")
        wgs_s = [const.tile([128, NH], BF16, tag=f"wgs_s{l}", name=f"wgs_s{l}")
                 for l in range(L)]
        wgs_n = [const.tile([128, NH], BF16, tag=f"wgs_n{l}", name=f"wgs_n{l}")
                 for l in range(L)]
        bgs = const.tile([128, L], F32, tag="bgs")
        wih = [const.tile([128, 4 * NH], BF16, tag=f"wih{l}", name=f"wih{l}")
               for l in range(L)]
        whh = [const.tile([128, 4 * NH], BF16, tag=f"whh{l}", name=f"whh{l}")
               for l in range(L)]
        blstm = const.tile([128, 8], F32, tag="blstm")
        blr = const.tile([1, 8 * NH], BF16, tag="blr")
        w_emb = [const.tile([128, NHE], BF16, tag=f"w_emb{t}", name=f"w_emb{t}")
                 for t in range(2)]
        wfc_aa = const.tile([128, 128], BF16, tag="wfc_aa")
        wfc_ba = const.tile([64, 128], BF16, tag="wfc_ba")
        wfc_ab = const.tile([128, 64], BF16, tag="wfc_ab")
        wfc_bb = const.tile([64, 64], BF16, tag="wfc_bb")
        w_out_a = const.tile([128, NOUT], BF16, tag="w_out_a")
        w_out_b = const.tile([64, NOUT], BF16, tag="w_out_b")
        bout_col = const.tile([NOUT, 1], F32, tag="bout_col")
        small = {}
        for nm, p in [("sc_in", NH), ("sh_in", NH), ("sc_in_h", NH),
                      ("sh_in2", NH), ("sc_emb", NHE), ("sh_emb", NHE),
                      ("sc_fc_a", 128), ("sh_fc_a", 128),
                      ("sc_fc_b", 64), ("sh_fc_b", 64)]:
            small[nm] = const.tile([p, 1], F32, tag=nm, name=nm)
        embin = const.tile([128, 2, PC], BF16, tag="embin")

        ident_bf = const.tile([128, 128], BF16, tag="ident")
        make_identity(nc, ident_bf)
        ident20 = const.tile([NOUT, NOUT], F32, tag="ident20")
        make_identity(nc, ident20)
        # pre-sync dummy #0: queued first on the CC stream, runs in the
        # dead window right after the init barrier
        nc.scalar.dma_start(dmyb[0].ap(), ident_bf[:16, :])
        nc.gpsimd.collective_compute(
            "AllGather", OP.bypass, replica_groups=groups,
            ins=[dmyb[0].ap().opt()], outs=[dmyg[0].ap().opt()])
        ones_col_bf = const.tile([128, 1], BF16, tag="ones_col")
        nc.vector.memset(ones_col_bf, 1.0)
        ones_row = const.tile([1, 128], F32, tag="ones_row")
        nc.vector.memset(ones_row, 1.0)
        ones20_bf = const.tile([NOUT, 1], BF16, tag="ones20")
        nc.vector.memset(ones20_bf, 1.0)
        ones_r20 = const.tile([1, NOUT], F32, tag="ones_r20")
        nc.vector.memset(ones_r20, 1.0)
        onesw = const.tile([1, 512], BF16, tag="onesw")
        nc.vector.memset(onesw, 1.0)

        # ---- sync/SP ring, in consumption order: w_in, x, emb, consts, adj
        nc.sync.dma_start(w_in.rearrange("p (t j) -> p t j", t=FT),
                          d_w_inT.ap().rearrange("(t p) j -> p t j", p=128))
        xq = []
        for e in range(NE):
            t = px.tile([128, FT, EW], BF16, tag="xq", name=f"xq{e}")
            nc.sync.dma_start(t, d_x[e])
            xq.append(t)
        nc.sync.dma_start(embin, d_emb.ap())
        # consts go on the scalar (ACT) HWDGE ring so they never stall the
        # bulk x/adjacency stream on the sync ring
        for l in range(L):
            nc.scalar.dma_start(wgs_s[l], d_wgs_sT[l])
            nc.scalar.dma_start(wgs_n[l], d_wgs_nT[l])
            nc.scalar.dma_start(wih[l], d_wihT[l])
            nc.scalar.dma_start(whh[l], d_whhT[l])
        nc.scalar.dma_start(bgs, d_bgs.ap())
        nc.scalar.dma_start(blstm, d_blstm.ap())
        nc.scalar.dma_start(blr, d_blr.ap())
        for t in range(2):
            nc.scalar.dma_start(w_emb[t], d_w_embT[t * 128:(t + 1) * 128, :])
        nc.scalar.dma_start(wfc_aa, d_w_fcT[:128, :128])
        nc.scalar.dma_start(wfc_ba, d_w_fcT[128:, :128])
        nc.scalar.dma_start(wfc_ab, d_w_fcT[:128, 128:])
        nc.scalar.dma_start(wfc_bb, d_w_fcT[128:, 128:])
        nc.scalar.dma_start(w_out_a, d_w_outT[:128, :])
        nc.scalar.dma_start(w_out_b, d_w_outT[128:, :])
        nc.scalar.dma_start(bout_col, d_bout.ap())
        for nm in small:
            nc.scalar.dma_start(small[nm], d_sm[nm].ap())
        # adjacency: h=0 half of every chunk first (agg0 consumption order),
        # then the h=1 halves
        adjres = [padjr.tile([128, 2 * HT, PC], FP8, tag=f"adjres{g}",
                             name=f"adjres{g}") for g in range(NRES)]
        stream = {}
        for h in range(2):
            for g in range(NC):
                if g < NRES:
                    nc.sync.dma_start(adjres[g][:, h * HT:(h + 1) * HT, :],
                                      d_adj[h, g])
                else:
                    t = padjs.tile([128, HT, PC], FP8, tag="adjst",
                                   name=f"st0_{h}_{g}")
                    nc.sync.dma_start(t, d_adj[h, g])
                    stream[(0, h, g)] = t

        # persistent activations (bf16; LSTM h-states are kept DOUBLED)
        hbf = [persist.tile([128, PC], BF16, tag=f"hbf{l}", name=f"hbf{l}")
               for l in range(3)]
        c_st = [persist.tile([128, PC], BF16, tag=f"c{l}", name=f"c{l}")
                for l in range(2)]
        o_bf = [persist.tile([128, PC], BF16, tag=f"o{t}", name=f"o{t}")
                for t in range(2)]
        p_bf = [persist.tile([128, PC], BF16, tag=f"p{t}", name=f"p{t}")
                for t in range(2)]
        eT = persist.tile([64, PC], BF16, tag="eT")
        hpost = persist.tile([128, PC], BF16, tag="hpost")
        hfca = persist.tile([128, PC], BF16, tag="hfca")
        hfcb = persist.tile([64, PC], BF16, tag="hfcb")
        out_sb = persist.tile([NOUT, PC], F32, tag="out_sb")
        outall = persist.tile([128, IT * NOUT], F32, tag="outall")

        # tmps
        tnb = ptmp.tile([128, PC], BF16, tag="tnb")      # neighbors / h-sum
        trl = ptmp.tile([128, PC], F32, tag="trl")       # gs relu out
        tsq = ptmp.tile([128, PC], BF16, tag="tsq")      # squared
        t3a = ptmp.tile([1, PC], F32, tag="t3a")         # norm / lse
        t3b = ptmp.tile([1, PC], F32, tag="t3b")         # 1/norm
        ty = ptmp.tile([NOUT, PC], F32, tag="ty")        # logits+bias
        # gate activations for one cell, all 4 gates, full width (bf16)
        gaq = pg.tile([128, 4, PC], BF16, tag="gaq")
        gtc = pg.tile([128, PC], BF16, tag="gtc")
        gfc = pg.tile([128, PC], BF16, tag="gfc")
        gig = pg.tile([128, PC], BF16, tag="gig")
        pe1 = pg.tile([128, PC], BF16, tag="pe1")   # post-pass elu scratch
        pe2 = pg.tile([128, PC], BF16, tag="pe2")
        tex = pg.tile([NOUT, PC], BF16, tag="tex")

        hnat = {}

        def elu_chunk(dst, src_ap, sc, sh, ytmp, etmp):
            """dst = elu(sc*src + sh); src may be PSUM."""
            nc.vector.tensor_scalar(ytmp, src_ap, sc, sh, OP.mult, OP.add)
            nc.scalar.activation(etmp, src_ap, AF.Exp, bias=sh, scale=sc)
            nc.vector.tensor_scalar(etmp, etmp, 1.0, -1.0, OP.min, OP.add)
            nc.vector.scalar_tensor_tensor(dst, ytmp, 0.0, etmp, OP.max, OP.add)

        with tc.tile_pool(name="psS", bufs=1, space="PSUM") as psS:

            def S(i):
                return psS.tile([128, 512], F32, tag=f"s{i}", name=f"s{i}")

            hn0 = pnat.tile([128, NC, PC], FP8, tag="hnat0", name="hnat0")

            def load_hnat(l, h):
                nc.gpsimd.dma_start(
                    hn0[:, :, h * 640:(h + 1) * 640],
                    hg[l][h].ap().rearrange("c p f -> p c f"))
                hnat[(l, h)] = hn0

            def gather(l, h, src_bf, tpool, defer_hnat=False):
                """transpose h-half of src to node-major fp8, AllGather.
                Lands in the h-half slice of the shared hnat tile (layer 1
                overwrites layer 0's halves after agg0 is done with them)."""
                tp = tpool.tile([128, HT * 128], BF16, tag="tp", name="tp")
                for s in range(HT):
                    col = h * 640 + s * 128
                    nc.tensor.transpose(tp[:, s * 128:(s + 1) * 128],
                                        src_bf[:, col:col + 128], ident_bf)
                loc = ploc.tile([128, HT * 128], FP8, tag="loc", name="loc")
                nc.vector.tensor_copy(loc, tp)
                nc.scalar.dma_start(bounce[l][h].ap(), loc)
                nc.gpsimd.collective_compute(
                    "AllGather", OP.bypass, replica_groups=groups,
                    ins=[bounce[l][h].ap().opt()],
                    outs=[hg[l][h].ap().opt()],
                )
                if not defer_hnat:
                    load_hnat(l, h)

            def lstm_chunk(l, t, ci, xin, hprev, c_tile, out_tile):
                """tanh-only LSTM t=0 cell chunk (gate scales pre-folded
                into the weights host-side); c and h states are DOUBLED."""
                o, w = CHUNKS[ci]
                gates = (0, 2, 3)
                gps = {}
                for g in gates:
                    ps = S(g)
                    nc.tensor.matmul(ps[:, :w],
                                     wih[l][:, g * 128:(g + 1) * 128],
                                     xin[:, o:o + w],
                                     start=True, stop=True)
                    gps[g] = ps
                for g in gates:
                    nc.scalar.activation(gaq[:, g, o:o + w], gps[g][:, :w],
                                         AF.Tanh,
                                         bias=blstm[:, l * 4 + g:l * 4 + g + 1])
                cs = c_tile[:, o:o + w]
                # c2 = (i~+1)*tanh(g)
                nc.vector.scalar_tensor_tensor(cs, gaq[:, 0, o:o + w], 1.0,
                                               gaq[:, 2, o:o + w], OP.add,
                                               OP.mult)
                nc.scalar.activation(gtc[:, o:o + w], cs, AF.Tanh, scale=0.5)
                # h2 = (o~+1)*tanh(c)
                nc.vector.scalar_tensor_tensor(out_tile[:, o:o + w],
                                               gaq[:, 3, o:o + w], 1.0,
                                               gtc[:, o:o + w],
                                               OP.add, OP.mult)

            with tc.tile_pool(name="psB", bufs=1, space="PSUM") as psB, \
                 tc.tile_pool(name="psT", bufs=1, space="PSUM") as psT:

                # ---- input projection (eighths) + gather0 per half ----
                for half in range(2):
                    for q in range(2):
                        for j in range(2):
                            e = half * 4 + q * 2 + j
                            ps = S(j)
                            for t in range(FT):
                                nc.tensor.matmul(
                                    ps[:, :EW],
                                    w_in[:, t * 128:(t + 1) * 128],
                                    xq[e][:, t, :],
                                    start=(t == 0), stop=(t == FT - 1))
                            qs = slice(j * EW, (j + 1) * EW)
                            nc.vector.tensor_scalar(gaq[:, 0, qs], ps[:, :EW],
                                                    small["sc_in"],
                                                    small["sh_in"],
                                                    OP.mult, OP.add)
                            nc.scalar.activation(gaq[:, 1, qs], ps[:, :EW],
                                                 AF.Exp, bias=small["sh_in"],
                                                 scale=small["sc_in"])
                        qw = slice(0, 2 * EW)
                        dst = hbf[0][:, (half * 4 + q * 2) * EW:
                                     (half * 4 + q * 2 + 2) * EW]
                        nc.vector.tensor_scalar(gaq[:, 1, qw], gaq[:, 1, qw],
                                                1.0, -1.0, OP.min, OP.add)
                        nc.vector.scalar_tensor_tensor(dst, gaq[:, 0, qw],
                                                       0.0, gaq[:, 1, qw],
                                                       OP.max, OP.add)
                    gather(0, half, hbf[0], psT)

                # ---- embed projection (during gather0 wait) ----
                for ci, (o, w) in enumerate(CHUNKS):
                    ps = S(2 + ci % 2)
                    nc.tensor.matmul(ps[:64, :w], w_emb[0],
                                     embin[:, 0, o:o + w], start=True,
                                     stop=False)
                    nc.tensor.matmul(ps[:64, :w], w_emb[1],
                                     embin[:, 1, o:o + w], start=False,
                                     stop=True)
                    elu_chunk(eT[:, o:o + w], ps[:64, :w], small["sc_emb"],
                              small["sh_emb"], gfc[:64, :w], gig[:64, :w])

                # ---- GNN layers ----
                for l in range(L):
                    # aggregation over all 80 k-tiles (both gather halves)
                    ps_agg = psB.tile([128, PC], F32, tag="agg", name="agg")
                    for h in range(2):
                        hn = hnat[(l, h)]
                        for g in range(NC):
                            for s in range(HT):
                                if g < NRES:
                                    rhs3 = adjres[g][:, h * HT + s, :]
                                else:
                                    rhs3 = stream[(l, h, g)][:, s, :]
                                ks = h * HT + s
                                lhsT = hn[:, g, ks * 128:(ks + 1) * 128]
                                for (o, w) in CHUNKS:
                                    nc.tensor.matmul(
                                        ps_agg[:, o:o + w], lhsT,
                                        rhs3[:, o:o + w],
                                        start=(h == 0 and g == 0 and s == 0),
                                        stop=(h == 1 and g == NC - 1
                                              and s == HT - 1))
                    # stream next layer's non-resident slabs (layer 0 only)
                    if l == 0:
                        for h in range(2):
                            for g in range(NRES, NC):
                                t = padjs.tile([128, HT, PC], FP8, tag="adjst",
                                               name=f"st1_{h}_{g}")
                                nc.sync.dma_start(t, d_adj[h, g])
                                stream[(1, h, g)] = t

                    # gs linear + relu + squared-sum; chunks pipeline across
                    # three psum tags, norm sums land in the (free) agg banks
                    sums_t = psB.tile([128, PC], F32, tag="agg",
                                      name=f"sums{l}")
                    for ci, (o, w) in enumerate(CHUNKS):
                        nc.vector.tensor_copy(tnb[:, o:o + w],
                                              ps_agg[:, o:o + w])
                        if l == 0 and ci == 0:
                            # pre-sync dummy #1: re-align cores right after
                            # agg0 so gather1's AllGathers chain cleanly
                            nc.scalar.dma_start(dmyb[1].ap(), tnb[:16, :128])
                            nc.gpsimd.collective_compute(
                                "AllGather", OP.bypass, replica_groups=groups,
                                ins=[dmyb[1].ap().opt()],
                                outs=[dmyg[1].ap().opt()])
                        ps_gs = S(ci)
                        nc.tensor.matmul(ps_gs[:, :w], wgs_s[l],
                                         hbf[l][:, o:o + w],
                                         start=True, stop=False)
                        nc.tensor.matmul(ps_gs[:, :w], wgs_n[l],
                                         tnb[:, o:o + w],
                                         start=False, stop=True)
                        nc.scalar.activation(trl[:, o:o + w], ps_gs[:, :w],
                                             AF.Relu, bias=bgs[:, l:l + 1])
                        nc.vector.tensor_tensor(tsq[:, o:o + w],
                                                trl[:, o:o + w],
                                                trl[:, o:o + w], OP.mult)
                        nc.tensor.matmul(sums_t[:1, o:o + w], ones_col_bf,
                                         tsq[:, o:o + w], start=True,
                                         stop=True)
                    # single sqrt over all chunks (one table load)
                    nc.scalar.activation(t3a[:, :PC], sums_t[:1, :PC],
                                         AF.Sqrt)
                    nc.vector.tensor_scalar_max(t3a[:, :PC], t3a[:, :PC],
                                                1e-12)
                    nc.vector.reciprocal(t3b[:, :PC], t3a[:, :PC])
                    for ci, (o, w) in enumerate(CHUNKS):
                        ps_bc = S(ci)
                        nc.tensor.matmul(ps_bc[:, :w], ones_row,
                                         t3b[:1, o:o + w],
                                         start=True, stop=True)
                        nc.vector.tensor_tensor(hbf[l + 1][:, o:o + w],
                                                trl[:, o:o + w],
                                                ps_bc[:, :w], OP.mult)
                        if l == 0 and ci == 1:
                            gather(1, 0, hbf[1], psT)
                        if l == 0 and ci == 2:
                            gather(1, 1, hbf[1], psT)

                    # LSTM t=0 cells run under layer-1 aggregation
                    if l == 0:
                        for ci in range(3):
                            lstm_chunk(0, 0, ci, hbf[1], None, c_st[0], o_bf[0])
                        for ci in range(3):
                            lstm_chunk(1, 0, ci, o_bf[0], None, c_st[1], p_bf[0])

        # ---- tail: LSTM t=1 (batched-gate, full-width), then Exp pass ----
        # (psS/psB/psT are closed here, freeing banks for gq + pp)
        with tc.tile_pool(name="psO", bufs=1, space="PSUM") as psO:

            def lstm_t1(l, xin, hprev, c_tile, out_tile):
                """t=1 cell: gates chunked through one 4-bank psum with the
                bias added by rank-1 matmuls, one Tanh per chunk for all 4
                gates, then full-width bf16 state updates."""
                gq = psO.tile([128, 4 * 512], F32, tag="gq", name=f"gq{l}")
                gq3 = gq.rearrange("p (g c) -> p g c", g=4)
                for ci, (o, w) in enumerate(CHUNKS):
                    for g in range(4):
                        sl = slice(g * 512, g * 512 + w)
                        nc.tensor.matmul(gq[:, sl],
                                         wih[l][:, g * 128:(g + 1) * 128],
                                         xin[:, o:o + w],
                                         start=True, stop=False)
                        nc.tensor.matmul(gq[:, sl],
                                         whh[l][:, g * 128:(g + 1) * 128],
                                         hprev[:, o:o + w],
                                         start=False, stop=True)
                    for g in range(4):
                        nc.scalar.activation(
                            gaq[:, g, o:o + w], gq[:, g * 512:g * 512 + w],
                            AF.Tanh,
                            bias=blstm[:, l * 4 + g:l * 4 + g + 1])
                    cw = slice(o, o + w)
                    nc.vector.scalar_tensor_tensor(gfc[:, cw], gaq[:, 1, cw],
                                                   1.0, c_tile[:, cw],
                                                   OP.add, OP.mult)
                    nc.vector.scalar_tensor_tensor(gig[:, cw], gaq[:, 0, cw],
                                                   1.0, gaq[:, 2, cw],
                                                   OP.add, OP.mult)
                    nc.vector.scalar_tensor_tensor(c_tile[:, cw], gfc[:, cw],
                                                   0.5, gig[:, cw],
                                                   OP.mult, OP.add)
                    nc.scalar.activation(gtc[:, cw], c_tile[:, cw], AF.Tanh,
                                         scale=0.5)
                    nc.vector.scalar_tensor_tensor(out_tile[:, cw],
                                                   gaq[:, 3, cw], 1.0,
                                                   gtc[:, cw],
                                                   OP.add, OP.mult)

            lstm_t1(0, hbf[2], o_bf[0], c_st[0], o_bf[1])
            lstm_t1(1, o_bf[1], p_bf[0], c_st[1], p_bf[1])

            pp = psO.tile([128, PC], F32, tag="pp", name="pp")
            # JK mean of doubled h's (0.25 folded into sc_in_h), per chunk
            # so the fc matmuls start as soon as the first LSTM chunk lands
            for (o, w) in CHUNKS:
                cw = slice(o, o + w)
                nc.vector.tensor_tensor(trl[:, cw], p_bf[0][:, cw],
                                        p_bf[1][:, cw], OP.add)
                elu_chunk(hpost[:, cw], trl[:, cw], small["sc_in_h"],
                          small["sh_in2"], pe1[:, cw], pe2[:, cw])
            # fc on concat([hpost, eT]); the b-half goes through the (now
            # free) gq banks so both fc branches + elus run in parallel
            gqp = psO.tile([128, 4 * 512], F32, tag="gq", name="gq_post")
            for (o, w) in CHUNKS:
                nc.tensor.matmul(pp[:, o:o + w], wfc_aa, hpost[:, o:o + w],
                                 start=True, stop=False)
                nc.tensor.matmul(pp[:, o:o + w], wfc_ba, eT[:, o:o + w],
                                 start=False, stop=True)
            for (o, w) in CHUNKS:
                nc.tensor.matmul(gqp[:64, o:o + w], wfc_ab,
                                 hpost[:, o:o + w], start=True, stop=False)
                nc.tensor.matmul(gqp[:64, o:o + w], wfc_bb, eT[:, o:o + w],
                                 start=False, stop=True)
            elu_chunk(hfca, pp, small["sc_fc_a"], small["sh_fc_a"],
                      pe1, pe2)
            elu_chunk(hfcb, gqp[:64, :PC], small["sc_fc_b"],
                      small["sh_fc_b"], gtc[:64, :], gfc[:64, :])
            # logits (feature-major); |logits| < 3 so exp is safe
            for (o, w) in CHUNKS:
                nc.tensor.matmul(pp[:NOUT, o:o + w], w_out_a,
                                 hfca[:, o:o + w], start=True, stop=False)
                nc.tensor.matmul(pp[:NOUT, o:o + w], w_out_b,
                                 hfcb[:, o:o + w], start=False, stop=True)
            for (o, w) in CHUNKS:
                nc.vector.tensor_scalar(ty[:, o:o + w], pp[:NOUT, o:o + w],
                                        bout_col, None, OP.add)
                nc.scalar.activation(tex[:, o:o + w], pp[:NOUT, o:o + w],
                                     AF.Exp, bias=bout_col)
                nc.tensor.matmul(pp[:1, o:o + w], ones20_bf,
                                 tex[:, o:o + w], start=True, stop=True)
            nc.scalar.activation(t3a[:, :PC], pp[:1, :PC], AF.Ln)
            for (o, w) in CHUNKS:
                nc.tensor.matmul(pp[:NOUT, o:o + w], ones_r20,
                                 t3a[:1, o:o + w], start=True, stop=True)
                nc.vector.tensor_tensor(out_sb[:, o:o + w], ty[:, o:o + w],
                                        pp[:NOUT, o:o + w], OP.subtract)
                for it in range(o // 128, (o + w) // 128):
                    nc.tensor.transpose(
                        pp[:, it * NOUT:(it + 1) * NOUT],
                        out_sb[:, it * 128:(it + 1) * 128], ident20)
            nc.vector.tensor_copy(outall, pp[:, :IT * NOUT])
            nc.sync.dma_start(d_out.ap(), outall)

    nc.compile()
    return nc


# --------------------------------------------------------------------------
# host side
# --------------------------------------------------------------------------

def _stage_inputs(
    x, embed, adj, W_in, b_in, bn_in_g, bn_in_b, bn_in_rm, bn_in_rv,
    W_gs, b_gs, Wih0, Whh0, bih0, bhh0, Wih1, Whh1, bih1, bhh1,
    W_emb, b_emb, bn_emb_g, bn_emb_b, bn_emb_rm, bn_emb_rv,
    W_fc, b_fc, bn_fc_g, bn_fc_b, bn_fc_rm, bn_fc_rv, W_out, b_out,
):
    x = np.asarray(x, np.float32)
    embed = np.asarray(embed, np.float32)
    adj = np.asarray(adj, np.float32)

    w_inT = np.zeros((FPAD, NH), ml_dtypes.bfloat16)
    w_inT[:NFEAT] = _bf(np.asarray(W_in, np.float32).T)

    def bn_fold(g, b, rm, rv, lin_b=None):
        g = np.asarray(g, np.float32); b = np.asarray(b, np.float32)
        rm = np.asarray(rm, np.float32); rv = np.asarray(rv, np.float32)
        sc = g / np.sqrt(rv + BN_EPS)
        base = lin_b if lin_b is not None else 0.0
        shv = sc * (base - rm) + b
        return _f32(sc), _f32(shv)

    sc_in, sh_in = bn_fold(bn_in_g, bn_in_b, bn_in_rm, bn_in_rv,
                           np.asarray(b_in, np.float32))
    _, sh_in2 = bn_fold(bn_in_g, bn_in_b, bn_in_rm, bn_in_rv)
    sc_emb, sh_emb = bn_fold(bn_emb_g, bn_emb_b, bn_emb_rm, bn_emb_rv,
                             np.asarray(b_emb, np.float32))
    sc_fc, sh_fc = bn_fold(bn_fc_g, bn_fc_b, bn_fc_rm, bn_fc_rv,
                           np.asarray(b_fc, np.float32))

    W_gs = np.asarray(W_gs, np.float32)
    wgs_sT = _bf(np.stack([W_gs[l][:, :NH].T for l in range(L)]))
    wgs_nT = _bf(np.stack([W_gs[l][:, NH:].T for l in range(L)])
                 * (1.0 / ADJ_SCALE))
    bgs = _f32(np.asarray(b_gs, np.float32).T)          # [NH, L]

    # tanh-only LSTM: layer-1 inputs and all h_prev are DOUBLED h states,
    # so Wih1 and both Whh get 0.5 folded in; additionally the sigmoid
    # gates (i/f/o) are computed as tanh(z/2+b/2), so those gate blocks of
    # W and b get another 0.5 -- activations then all run at scale=1.
    wih_l = [np.asarray(Wih0, np.float32).T.copy(),
             0.5 * np.asarray(Wih1, np.float32).T]
    whh_l = [0.5 * np.asarray(Whh0, np.float32).T,
             0.5 * np.asarray(Whh1, np.float32).T]
    for arr in wih_l + whh_l:
        for g in (0, 1, 3):
            arr[:, g * NH:(g + 1) * NH] *= 0.5
    wihT = np.stack([_bf(wih_l[0]), _bf(wih_l[1])])
    whhT = np.stack([_bf(whh_l[0]), _bf(whh_l[1])])
    bl = np.stack([np.asarray(bih0, np.float32) + np.asarray(bhh0, np.float32),
                   np.asarray(bih1, np.float32) + np.asarray(bhh1, np.float32)])
    blstm = np.zeros((NH, 8), np.float32)
    for l in range(2):
        for g in range(4):
            f = 1.0 if g == 2 else 0.5
            blstm[:, l * 4 + g] = f * bl[l][g * NH:(g + 1) * NH]
    # same biases as a row vector for the rank-1 bias matmuls (tail cells)
    blr = np.zeros((1, 8 * NH), ml_dtypes.bfloat16)
    for l in range(2):
        for g in range(4):
            blr[0, (l * 4 + g) * NH:(l * 4 + g + 1) * NH] = \
                _bf(blstm[:, l * 4 + g])

    shared = {
        "w_inT": w_inT,
        "wgs_sT": wgs_sT, "wgs_nT": wgs_nT, "bgs": bgs,
        "wihT": _bf(wihT), "whhT": _bf(whhT), "blstm": blstm, "blr": blr,
        "w_embT": _bf(np.asarray(W_emb, np.float32).T),
        "w_fcT": _bf(np.asarray(W_fc, np.float32).T),
        "w_outT": _bf(np.asarray(W_out, np.float32).T),
        "bout_col": _f32(np.asarray(b_out, np.float32))[:, None],
        "sc_in": sc_in[:, None], "sh_in": sh_in[:, None],
        # JK mean of two DOUBLED h states: 0.5 * 0.5 = 0.25
        "sc_in_h": _f32(0.25 * sc_in)[:, None], "sh_in2": sh_in2[:, None],
        "sc_emb": sc_emb[:, None], "sh_emb": sh_emb[:, None],
        "sc_fc_a": _f32(sc_fc[:128])[:, None], "sh_fc_a": _f32(sh_fc[:128])[:, None],
        "sc_fc_b": _f32(sc_fc[128:])[:, None], "sh_fc_b": _f32(sh_fc[128:])[:, None],
    }

    rowsum = adj.sum(axis=1)                    # fp32, exact rows
    in_maps = []
    for c in range(NC):
        rows = slice(c * NPC, (c + 1) * NPC)
        scaled = adj[rows] * (ADJ_SCALE / rowsum[rows])[:, None]
        at = scaled.T                           # [10000, 1250]
        padded = np.zeros((NP, PC), np.float32)
        for ck in range(NC):
            padded[ck * PC:ck * PC + NPC, :NPC] = at[ck * NPC:(ck + 1) * NPC]
        adj8 = padded.astype(ml_dtypes.float8_e4m3fn)
        # [t, p, i] -> [h, g, p, s, i] with t = g*10 + h*5 + s
        adj8 = np.ascontiguousarray(
            adj8.reshape(NC, 2, HT, 128, PC).transpose(1, 0, 3, 2, 4))

        xT = np.zeros((FPAD, PC), ml_dtypes.bfloat16)
        xT[:NFEAT, :NPC] = _bf(x[rows].T)
        x8 = np.ascontiguousarray(
            xT.reshape(FT, 128, NE, EW).transpose(2, 1, 0, 3))

        embT = np.zeros((NFE, PC), ml_dtypes.bfloat16)
        embT[:, :NPC] = _bf(embed[rows].T)
        embT = np.ascontiguousarray(embT.reshape(2, 128, PC).transpose(1, 0, 2))

        m = {"adj8": adj8, "x8": x8, "embT": embT}
        m.update(shared)
        in_maps.append(m)
    return in_maps


def kernel(**inputs) -> np.ndarray:
    global _CACHED_NC, LAST_RESULT
    in_maps = _stage_inputs(**inputs)
    if _CACHED_NC is None:
        _CACHED_NC = _build_program()
    nc = _CACHED_NC
    trace = bool(int(os.environ.get("GSAGE_TRACE", "0")))
    res = run_bass_kernel_spmd(
        nc, in_maps, core_ids=list(range(NC)), trace=trace,
    )
    LAST_RESULT = res
    parts = []
    for c in range(NC):
        o = np.asarray(res.results[c]["out"], np.float32)
        o = o.reshape(128, IT, NOUT).transpose(1, 0, 2).reshape(PC, NOUT)
        parts.append(o[:NPC])
    out = np.concatenate(parts, axis=0)
    return np.ascontiguousarray(out, np.float32)


if __name__ == "__main__":
    import reference
    inputs = reference.setup_inputs()
    out = kernel(**{k: np.asarray(v) for k, v in inputs.items()})
    print("out", out.shape, out.dtype)


# revision 55
# speedup vs baseline: 1.0547x; 1.0547x over previous
"""GraphSAGE (gnn_message_passing) forward pass on 8 Trainium2 NeuronCores.

Sharding (hardcoded): row-shard the 10000 nodes across 8 cores (1250 each,
padded to 1280).  The row-normalized adjacency shard is staged host-side as
fp8e4m3 ([10240, 1280] transposed, scaled by 4096 with the inverse scale
folded into W_neigh) and loaded into SBUF once -- both GNN layers aggregate
from the same resident/streamed copy.  Node features travel between layers
via fp8 AllGathers (two halves each, pipelined against the aggregation
matmuls).  Small weights / LSTM params are replicated.

The LSTM is computed in "tanh-only" form (sigmoid(x) = 0.5*tanh(x/2)+0.5,
with the 0.5 factors folded into Whh/Wih1/biases and cell/h states kept
doubled) so the scalar engine never swaps activation tables inside the
recurrence; elu/softmax stages are likewise grouped by activation function
(activation-table loads cost ~1.3us each).
"""

import os
from contextlib import ExitStack

import numpy as np
import ml_dtypes

import concourse.bass as bass
import concourse.bacc as bacc
import concourse.mybir as mybir
import concourse.tile as tile
from concourse.bass_utils import run_bass_kernel_spmd
from concourse.masks import make_identity

F32 = mybir.dt.float32
BF16 = mybir.dt.bfloat16
FP8 = mybir.dt.float8e4
AX = mybir.AxisListType
OP = mybir.AluOpType
AF = mybir.ActivationFunctionType

# ---- problem constants (hardcoded per spec) ----
N = 10000        # nodes
NC = 8           # cores
NPC = 1250       # original nodes per core
PC = 1280        # padded nodes per core
NP = NC * PC     # padded total nodes = 10240
KT = NP // 128   # 80 contraction tiles
IT = PC // 128   # 10 node tiles per core
HT = 5           # k-tiles per gather half per core
NFEAT = 2000
FPAD = 2048
FT = FPAD // 128  # 16
NH = 128
NHE = 64
NFE = 256
D = NH + NHE     # 192
NOUT = 20
L = 2
BN_EPS = 1e-5
ADJ_SCALE = 4096.0
NRES = 4         # adjacency chunks resident in SBUF across both layers
NE = 8           # x eighths
EW = PC // NE    # 160

CHUNKS = [(0, 512), (512, 512), (1024, 256)]

LAST_RESULT = None  # test.py reads exec_time info from here

_CACHED_NC = None


def _bf(a):
    return np.asarray(a, dtype=ml_dtypes.bfloat16)


def _f32(a):
    return np.ascontiguousarray(a, dtype=np.float32)


# --------------------------------------------------------------------------
# device program
# --------------------------------------------------------------------------

def _build_program():
    nc = bacc.Bacc("TRN2", target_bir_lowering=False, debug=False, num_devices=NC)

    def inp(name, shape, dtype):
        return nc.declare_dram_parameter(name, list(shape), dtype, isOutput=False)

    # per-core tensors
    d_adj = inp("adj8", [2, NC, 128, HT, PC], FP8)   # [half, chunk, p, s, i]
    d_x = inp("x8", [NE, 128, FT, EW], BF16)
    d_emb = inp("embT", [128, 2, PC], BF16)
    # replicated weights
    d_w_inT = inp("w_inT", [FPAD, NH], BF16)
    d_wgs_sT = inp("wgs_sT", [L, NH, NH], BF16)
    d_wgs_nT = inp("wgs_nT", [L, NH, NH], BF16)      # pre-scaled by 1/ADJ_SCALE
    d_bgs = inp("bgs", [NH, L], F32)
    d_wihT = inp("wihT", [L, NH, 4 * NH], BF16)      # gate scales pre-folded
    d_whhT = inp("whhT", [L, NH, 4 * NH], BF16)      # gate scales pre-folded
    d_blstm = inp("blstm", [NH, 2 * 4], F32)         # i/f/o pre-scaled by 0.5
    d_blr = inp("blr", [1, 8 * NH], BF16)            # same, as row vector
    d_w_embT = inp("w_embT", [NFE, NHE], BF16)
    d_w_fcT = inp("w_fcT", [D, D], BF16)
    d_w_outT = inp("w_outT", [D, NOUT], BF16)
    d_bout = inp("bout_col", [NOUT, 1], F32)
    d_sm = {}
    for nm, p in [("sc_in", NH), ("sh_in", NH), ("sc_in_h", NH), ("sh_in2", NH),
                  ("sc_emb", NHE), ("sh_emb", NHE),
                  ("sc_fc_a", 128), ("sh_fc_a", 128),
                  ("sc_fc_b", 64), ("sh_fc_b", 64)]:
        d_sm[nm] = inp(nm, [p, 1], F32)
    d_out = nc.declare_dram_parameter("out", [128, IT * NOUT], F32, isOutput=True)

    # internal DRAM for collectives: both layers gather in halves; the
    # second half's AllGather chains on the CC stream and hides under the
    # first half's aggregation matmuls.
    bounce = [[nc.dram_tensor(f"bounce{l}_{h}", [128, HT * 128], FP8)
               for h in range(2)] for l in range(L)]
    hg = [[nc.dram_tensor(f"hg{l}_{h}", [NC, 128, HT * 128], FP8,
                          addr_space="Shared") for h in range(2)]
          for l in range(L)]
    # tiny pre-sync AllGathers: absorb cross-core skew on the CC stream
    # just before each real gather batch, so the real ones chain at data
    # speed instead of paying the straggler wait themselves
    dmyb = [nc.dram_tensor(f"dmyb{i}", [16, 128], BF16) for i in range(2)]
    dmyg = [nc.dram_tensor(f"dmyg{i}", [NC, 16, 128], BF16,
                           addr_space="Shared") for i in range(2)]
    groups = [list(range(NC))]

    with tile.TileContext(nc) as tc, ExitStack() as top:
        const = top.enter_context(tc.tile_pool(name="const", bufs=1))
        persist = top.enter_context(tc.tile_pool(name="persist", bufs=1))
        padjr = top.enter_context(tc.tile_pool(name="adjr", bufs=1))
        padjs = top.enter_context(tc.tile_pool(name="adjs", bufs=3))
        px = top.enter_context(tc.tile_pool(name="px", bufs=2))
        pnat = top.enter_context(tc.tile_pool(name="pnat", bufs=2))
        ploc = top.enter_context(tc.tile_pool(name="ploc", bufs=2))
        ptmp = top.enter_context(tc.tile_pool(name="ptmp", bufs=1))
        pg = top.enter_context(tc.tile_pool(name="pg", bufs=1))

        # ---- tiles for constants (DMAs issued in ring-order below) ----
        w_in = const.tile([128, FPAD], BF16, tag="w_in")
        wgs_s = [const.tile([128, NH], BF16, tag=f"wgs_s{l}", name=f"wgs_s{l}")
                 for l in range(L)]
        wgs_n = [const.tile([128, NH], BF16, tag=f"wgs_n{l}", name=f"wgs_n{l}")
                 for l in range(L)]
        bgs = const.tile([128, L], F32, tag="bgs")
        wih = [const.tile([128, 4 * NH], BF16, tag=f"wih{l}", name=f"wih{l}")
               for l in range(L)]
        whh = [const.tile([128, 4 * NH], BF16, tag=f"whh{l}", name=f"whh{l}")
               for l in range(L)]
        blstm = const.tile([128, 8], F32, tag="blstm")
        blr = const.tile([1, 8 * NH], BF16, tag="blr")
        w_emb = [const.tile([128, NHE], BF16, tag=f"w_emb{t}", name=f"w_emb{t}")
                 for t in range(2)]
        wfc_aa = const.tile([128, 128], BF16, tag="wfc_aa")
        wfc_ba = const.tile([64, 128], BF16, tag="wfc_ba")
        wfc_ab = const.tile([128, 64], BF16, tag="wfc_ab")
        wfc_bb = const.tile([64, 64], BF16, tag="wfc_bb")
        w_out_a = const.tile([128, NOUT], BF16, tag="w_out_a")
        w_out_b = const.tile([64, NOUT], BF16, tag="w_out_b")
        bout_col = const.tile([NOUT, 1], F32, tag="bout_col")
        small = {}
        for nm, p in [("sc_in", NH), ("sh_in", NH), ("sc_in_h", NH),
                      ("sh_in2", NH), ("sc_emb", NHE), ("sh_emb", NHE),
                      ("sc_fc_a", 128), ("sh_fc_a", 128),
                      ("sc_fc_b", 64), ("sh_fc_b", 64)]:
            small[nm] = const.tile([p, 1], F32, tag=nm, name=nm)
        embin = const.tile([128, 2, PC], BF16, tag="embin")

        ident_bf = const.tile([128, 128], BF16, tag="ident")
        make_identity(nc, ident_bf)
        ident20 = const.tile([NOUT, NOUT], F32, tag="ident20")
        make_identity(nc, ident20)
        # pre-sync dummy #0: queued first on the CC stream, runs in the
        # dead window right after the init barrier
        nc.scalar.dma_start(dmyb[0].ap(), ident_bf[:16, :])
        nc.gpsimd.collective_compute(
            "AllGather", OP.bypass, replica_groups=groups,
            ins=[dmyb[0].ap().opt()], outs=[dmyg[0].ap().opt()])
        ones_col_bf = const.tile([128, 1], BF16, tag="ones_col")
        nc.vector.memset(ones_col_bf, 1.0)
        ones_row = const.tile([1, 128], F32, tag="ones_row")
        nc.vector.memset(ones_row, 1.0)
        ones20_bf = const.tile([NOUT, 1], BF16, tag="ones20")
        nc.vector.memset(ones20_bf, 1.0)
        ones_r20 = const.tile([1, NOUT], F32, tag="ones_r20")
        nc.vector.memset(ones_r20, 1.0)
        onesw = const.tile([1, 512], BF16, tag="onesw")
        nc.vector.memset(onesw, 1.0)

        # ---- sync/SP ring, in consumption order: w_in, x, emb, consts, adj
        nc.sync.dma_start(w_in.rearrange("p (t j) -> p t j", t=FT),
                          d_w_inT.ap().rearrange("(t p) j -> p t j", p=128))
        xq = []
        for e in range(NE):
            t = px.tile([128, FT, EW], BF16, tag="xq", name=f"xq{e}")
            nc.sync.dma_start(t, d_x[e])
            xq.append(t)
        nc.sync.dma_start(embin, d_emb.ap())
        # consts go on the scalar (ACT) HWDGE ring so they never stall the
        # bulk x/adjacency stream on the sync ring
        for l in range(L):
            nc.scalar.dma_start(wgs_s[l], d_wgs_sT[l])
            nc.scalar.dma_start(wgs_n[l], d_wgs_nT[l])
            nc.scalar.dma_start(wih[l], d_wihT[l])
            nc.scalar.dma_start(whh[l], d_whhT[l])
        nc.scalar.dma_start(bgs, d_bgs.ap())
        nc.scalar.dma_start(blstm, d_blstm.ap())
        nc.scalar.dma_start(blr, d_blr.ap())
        for t in range(2):
            nc.scalar.dma_start(w_emb[t], d_w_embT[t * 128:(t + 1) * 128, :])
        nc.scalar.dma_start(wfc_aa, d_w_fcT[:128, :128])
        nc.scalar.dma_start(wfc_ba, d_w_fcT[128:, :128])
        nc.scalar.dma_start(wfc_ab, d_w_fcT[:128, 128:])
        nc.scalar.dma_start(wfc_bb, d_w_fcT[128:, 128:])
        nc.scalar.dma_start(w_out_a, d_w_outT[:128, :])
        nc.scalar.dma_start(w_out_b, d_w_outT[128:, :])
        nc.scalar.dma_start(bout_col, d_bout.ap())
        for nm in small:
            nc.scalar.dma_start(small[nm], d_sm[nm].ap())
        # adjacency: h=0 half of every chunk first (agg0 consumption order),
        # then the h=1 halves
        adjres = [padjr.tile([128, 2 * HT, PC], FP8, tag=f"adjres{g}",
                             name=f"adjres{g}") for g in range(NRES)]
        stream = {}
        for h in range(2):
            for g in range(NC):
                if g < NRES:
                    nc.sync.dma_start(adjres[g][:, h * HT:(h + 1) * HT, :],
                                      d_adj[h, g])
                else:
                    t = padjs.tile([128, HT, PC], FP8, tag="adjst",
                                   name=f"st0_{h}_{g}")
                    nc.sync.dma_start(t, d_adj[h, g])
                    stream[(0, h, g)] = t

        # persistent activations (bf16; LSTM h-states are kept DOUBLED)
        hbf = [persist.tile([128, PC], BF16, tag=f"hbf{l}", name=f"hbf{l}")
               for l in range(3)]
        c_st = [persist.tile([128, PC], BF16, tag=f"c{l}", name=f"c{l}")
                for l in range(2)]
        o_bf = [persist.tile([128, PC], BF16, tag=f"o{t}", name=f"o{t}")
                for t in range(2)]
        p_bf = [persist.tile([128, PC], BF16, tag=f"p{t}", name=f"p{t}")
                for t in range(2)]
        eT = persist.tile([64, PC], BF16, tag="eT")
        hpost = persist.tile([128, PC], BF16, tag="hpost")
        hfca = persist.tile([128, PC], BF16, tag="hfca")
        hfcb = persist.tile([64, PC], BF16, tag="hfcb")
        out_sb = persist.tile([NOUT, PC], F32, tag="out_sb")
        outall = persist.tile([128, IT * NOUT], F32, tag="outall")

        # tmps
        tnb = ptmp.tile([128, PC], BF16, tag="tnb")      # neighbors / h-sum
        trl = ptmp.tile([128, PC], F32, tag="trl")       # gs relu out
        tsq = ptmp.tile([128, PC], BF16, tag="tsq")      # squared
        t3a = ptmp.tile([1, PC], F32, tag="t3a")         # norm / lse
        t3b = ptmp.tile([1, PC], F32, tag="t3b")         # 1/norm
        ty = ptmp.tile([NOUT, PC], F32, tag="ty")        # logits+bias
        # gate activations for one cell, all 4 gates, full width (bf16)
        gaq = pg.tile([128, 4, PC], BF16, tag="gaq")
        gtc = pg.tile([128, PC], BF16, tag="gtc")
        gfc = pg.tile([128, PC], BF16, tag="gfc")
        gig = pg.tile([128, PC], BF16, tag="gig")
        pe1 = pg.tile([128, PC], BF16, tag="pe1")   # post-pass elu scratch
        pe2 = pg.tile([128, PC], BF16, tag="pe2")
        tex = pg.tile([NOUT, PC], BF16, tag="tex")

        hnat = {}

        def elu_chunk(dst, src_ap, sc, sh, ytmp, etmp):
            """dst = elu(sc*src + sh); src may be PSUM."""
            nc.vector.tensor_scalar(ytmp, src_ap, sc, sh, OP.mult, OP.add)
            nc.scalar.activation(etmp, src_ap, AF.Exp, bias=sh, scale=sc)
            nc.vector.tensor_scalar(etmp, etmp, 1.0, -1.0, OP.min, OP.add)
            nc.vector.scalar_tensor_tensor(dst, ytmp, 0.0, etmp, OP.max, OP.add)

        with tc.tile_pool(name="psS", bufs=1, space="PSUM") as psS:

            def S(i):
                return psS.tile([128, 512], F32, tag=f"s{i}", name=f"s{i}")

            hn0 = pnat.tile([128, NC, PC], FP8, tag="hnat0", name="hnat0")

            def load_hnat(l, h):
                # sync ring (HWDGE, ~idle at gather time): lower setup
                # latency than the gpsimd SWDGE path
                nc.sync.dma_start(
                    hn0[:, :, h * 640:(h + 1) * 640],
                    hg[l][h].ap().rearrange("c p f -> p c f"))
                hnat[(l, h)] = hn0

            def gather(l, h, src_bf, tpool, defer_hnat=False):
                """transpose h-half of src to node-major fp8, AllGather.
                Lands in the h-half slice of the shared hnat tile (layer 1
                overwrites layer 0's halves after agg0 is done with them)."""
                tp = tpool.tile([128, HT * 128], BF16, tag="tp", name="tp")
                for s in range(HT):
                    col = h * 640 + s * 128
                    nc.tensor.transpose(tp[:, s * 128:(s + 1) * 128],
                                        src_bf[:, col:col + 128], ident_bf)
                loc = ploc.tile([128, HT * 128], FP8, tag="loc", name="loc")
                nc.vector.tensor_copy(loc, tp)
                nc.scalar.dma_start(bounce[l][h].ap(), loc)
                nc.gpsimd.collective_compute(
                    "AllGather", OP.bypass, replica_groups=groups,
                    ins=[bounce[l][h].ap().opt()],
                    outs=[hg[l][h].ap().opt()],
                )
                if not defer_hnat:
                    load_hnat(l, h)

            def lstm_chunk(l, t, ci, xin, hprev, c_tile, out_tile):
                """tanh-only LSTM t=0 cell chunk (gate scales pre-folded
                into the weights host-side); c and h states are DOUBLED."""
                o, w = CHUNKS[ci]
                gates = (0, 2, 3)
                gps = {}
                for g in gates:
                    ps = S(g)
                    nc.tensor.matmul(ps[:, :w],
                                     wih[l][:, g * 128:(g + 1) * 128],
                                     xin[:, o:o + w],
                                     start=True, stop=True)
                    gps[g] = ps
                for g in gates:
                    nc.scalar.activation(gaq[:, g, o:o + w], gps[g][:, :w],
                                         AF.Tanh,
                                         bias=blstm[:, l * 4 + g:l * 4 + g + 1])
                cs = c_tile[:, o:o + w]
                # c2 = (i~+1)*tanh(g)
                nc.vector.scalar_tensor_tensor(cs, gaq[:, 0, o:o + w], 1.0,
                                               gaq[:, 2, o:o + w], OP.add,
                                               OP.mult)
                nc.scalar.activation(gtc[:, o:o + w], cs, AF.Tanh, scale=0.5)
                # h2 = (o~+1)*tanh(c)
                nc.vector.scalar_tensor_tensor(out_tile[:, o:o + w],
                                               gaq[:, 3, o:o + w], 1.0,
                                               gtc[:, o:o + w],
                                               OP.add, OP.mult)

            with tc.tile_pool(name="psB", bufs=1, space="PSUM") as psB, \
                 tc.tile_pool(name="psT", bufs=1, space="PSUM") as psT:

                # ---- input projection (eighths) + gather0 per half ----
                for half in range(2):
                    for q in range(2):
                        for j in range(2):
                            e = half * 4 + q * 2 + j
                            ps = S(j)
                            for t in range(FT):
                                nc.tensor.matmul(
                                    ps[:, :EW],
                                    w_in[:, t * 128:(t + 1) * 128],
                                    xq[e][:, t, :],
                                    start=(t == 0), stop=(t == FT - 1))
                            qs = slice(j * EW, (j + 1) * EW)
                            nc.vector.tensor_scalar(gaq[:, 0, qs], ps[:, :EW],
                                                    small["sc_in"],
                                                    small["sh_in"],
                                                    OP.mult, OP.add)
                            nc.scalar.activation(gaq[:, 1, qs], ps[:, :EW],
                                                 AF.Exp, bias=small["sh_in"],
                                                 scale=small["sc_in"])
                        qw = slice(0, 2 * EW)
                        dst = hbf[0][:, (half * 4 + q * 2) * EW:
                                     (half * 4 + q * 2 + 2) * EW]
                        nc.vector.tensor_scalar(gaq[:, 1, qw], gaq[:, 1, qw],
                                                1.0, -1.0, OP.min, OP.add)
                        nc.vector.scalar_tensor_tensor(dst, gaq[:, 0, qw],
                                                       0.0, gaq[:, 1, qw],
                                                       OP.max, OP.add)
                    gather(0, half, hbf[0], psT)

                # ---- embed projection (during gather0 wait) ----
                for ci, (o, w) in enumerate(CHUNKS):
                    ps = S(2 + ci % 2)
                    nc.tensor.matmul(ps[:64, :w], w_emb[0],
                                     embin[:, 0, o:o + w], start=True,
                                     stop=False)
                    nc.tensor.matmul(ps[:64, :w], w_emb[1],
                                     embin[:, 1, o:o + w], start=False,
                                     stop=True)
                    elu_chunk(eT[:, o:o + w], ps[:64, :w], small["sc_emb"],
                              small["sh_emb"], gfc[:64, :w], gig[:64, :w])

                # ---- GNN layers ----
                for l in range(L):
                    # aggregation over all 80 k-tiles (both gather halves)
                    ps_agg = psB.tile([128, PC], F32, tag="agg", name="agg")
                    for h in range(2):
                        hn = hnat[(l, h)]
                        for g in range(NC):
                            for s in range(HT):
                                if g < NRES:
                                    rhs3 = adjres[g][:, h * HT + s, :]
                                else:
                                    rhs3 = stream[(l, h, g)][:, s, :]
                                ks = h * HT + s
                                lhsT = hn[:, g, ks * 128:(ks + 1) * 128]
                                for (o, w) in CHUNKS:
                                    nc.tensor.matmul(
                                        ps_agg[:, o:o + w], lhsT,
                                        rhs3[:, o:o + w],
                                        start=(h == 0 and g == 0 and s == 0),
                                        stop=(h == 1 and g == NC - 1
                                              and s == HT - 1))
                    # stream next layer's non-resident slabs (layer 0 only)
                    if l == 0:
                        for h in range(2):
                            for g in range(NRES, NC):
                                t = padjs.tile([128, HT, PC], FP8, tag="adjst",
                                               name=f"st1_{h}_{g}")
                                nc.sync.dma_start(t, d_adj[h, g])
                                stream[(1, h, g)] = t

                    # gs linear + relu + squared-sum; chunks pipeline across
                    # three psum tags, norm sums land in the (free) agg banks
                    sums_t = psB.tile([128, PC], F32, tag="agg",
                                      name=f"sums{l}")
                    for ci, (o, w) in enumerate(CHUNKS):
                        nc.vector.tensor_copy(tnb[:, o:o + w],
                                              ps_agg[:, o:o + w])
                        if l == 0 and ci == 0:
                            # pre-sync dummy #1: re-align cores right after
                            # agg0 so gather1's AllGathers chain cleanly
                            nc.scalar.dma_start(dmyb[1].ap(), tnb[:16, :128])
                            nc.gpsimd.collective_compute(
                                "AllGather", OP.bypass, replica_groups=groups,
                                ins=[dmyb[1].ap().opt()],
                                outs=[dmyg[1].ap().opt()])
                        ps_gs = S(ci)
                        nc.tensor.matmul(ps_gs[:, :w], wgs_s[l],
                                         hbf[l][:, o:o + w],
                                         start=True, stop=False)
                        nc.tensor.matmul(ps_gs[:, :w], wgs_n[l],
                                         tnb[:, o:o + w],
                                         start=False, stop=True)
                        nc.scalar.activation(trl[:, o:o + w], ps_gs[:, :w],
                                             AF.Relu, bias=bgs[:, l:l + 1])
                        nc.vector.tensor_tensor(tsq[:, o:o + w],
                                                trl[:, o:o + w],
                                                trl[:, o:o + w], OP.mult)
                        nc.tensor.matmul(sums_t[:1, o:o + w], ones_col_bf,
                                         tsq[:, o:o + w], start=True,
                                         stop=True)
                    # single sqrt over all chunks (one table load)
                    nc.scalar.activation(t3a[:, :PC], sums_t[:1, :PC],
                                         AF.Sqrt)
                    nc.vector.tensor_scalar_max(t3a[:, :PC], t3a[:, :PC],
                                                1e-12)
                    nc.vector.reciprocal(t3b[:, :PC], t3a[:, :PC])
                    for ci, (o, w) in enumerate(CHUNKS):
                        ps_bc = S(ci)
                        nc.tensor.matmul(ps_bc[:, :w], ones_row,
                                         t3b[:1, o:o + w],
                                         start=True, stop=True)
                        nc.vector.tensor_tensor(hbf[l + 1][:, o:o + w],
                                                trl[:, o:o + w],
                                                ps_bc[:, :w], OP.mult)
                        if l == 0 and ci == 1:
                            gather(1, 0, hbf[1], psT)
                        if l == 0 and ci == 2:
                            gather(1, 1, hbf[1], psT)

                    # LSTM t=0 cells run under layer-1 aggregation
                    if l == 0:
                        for ci in range(3):
                            lstm_chunk(0, 0, ci, hbf[1], None, c_st[0], o_bf[0])
                        for ci in range(3):
                            lstm_chunk(1, 0, ci, o_bf[0], None, c_st[1], p_bf[0])

        # ---- tail: LSTM t=1 (batched-gate, full-width), then Exp pass ----
        # (psS/psB/psT are closed here, freeing banks for gq + pp)
        with tc.tile_pool(name="psO", bufs=1, space="PSUM") as psO:

            def lstm_t1(l, xin, hprev, c_tile, out_tile):
                """t=1 cell: gates chunked through one 4-bank psum with the
                bias added by rank-1 matmuls, one Tanh per chunk for all 4
                gates, then full-width bf16 state updates."""
                gq = psO.tile([128, 4 * 512], F32, tag="gq", name=f"gq{l}")
                gq3 = gq.rearrange("p (g c) -> p g c", g=4)
                for ci, (o, w) in enumerate(CHUNKS):
                    for g in range(4):
                        sl = slice(g * 512, g * 512 + w)
                        nc.tensor.matmul(gq[:, sl],
                                         wih[l][:, g * 128:(g + 1) * 128],
                                         xin[:, o:o + w],
                                         start=True, stop=False)
                        nc.tensor.matmul(gq[:, sl],
                                         whh[l][:, g * 128:(g + 1) * 128],
                                         hprev[:, o:o + w],
                                         start=False, stop=True)
                    for g in (1, 0, 2, 3):   # f first: its stt starts sooner
                        nc.scalar.activation(
                            gaq[:, g, o:o + w], gq[:, g * 512:g * 512 + w],
                            AF.Tanh,
                            bias=blstm[:, l * 4 + g:l * 4 + g + 1])
                    cw = slice(o, o + w)
                    nc.vector.scalar_tensor_tensor(gfc[:, cw], gaq[:, 1, cw],
                                                   1.0, c_tile[:, cw],
                                                   OP.add, OP.mult)
                    nc.vector.scalar_tensor_tensor(gig[:, cw], gaq[:, 0, cw],
                                                   1.0, gaq[:, 2, cw],
                                                   OP.add, OP.mult)
                    nc.vector.scalar_tensor_tensor(c_tile[:, cw], gfc[:, cw],
                                                   0.5, gig[:, cw],
                                                   OP.mult, OP.add)
                    nc.scalar.activation(gtc[:, cw], c_tile[:, cw], AF.Tanh,
                                         scale=0.5)
                    nc.vector.scalar_tensor_tensor(out_tile[:, cw],
                                                   gaq[:, 3, cw], 1.0,
                                                   gtc[:, cw],
                                                   OP.add, OP.mult)

            lstm_t1(0, hbf[2], o_bf[0], c_st[0], o_bf[1])
            lstm_t1(1, o_bf[1], p_bf[0], c_st[1], p_bf[1])

            pp = psO.tile([128, PC], F32, tag="pp", name="pp")
            # JK mean of doubled h's (0.25 folded into sc_in_h), per chunk
            # so the fc matmuls start as soon as the first LSTM chunk lands
            for (o, w) in CHUNKS:
                cw = slice(o, o + w)
                nc.vector.tensor_tensor(trl[:, cw], p_bf[0][:, cw],
                                        p_bf[1][:, cw], OP.add)
                elu_chunk(hpost[:, cw], trl[:, cw], small["sc_in_h"],
                          small["sh_in2"], pe1[:, cw], pe2[:, cw])
            # fc on concat([hpost, eT]); the b-half goes through the (now
            # free) gq banks so both fc branches + elus run in parallel
            gqp = psO.tile([128, 4 * 512], F32, tag="gq", name="gq_post")
            for (o, w) in CHUNKS:
                nc.tensor.matmul(pp[:, o:o + w], wfc_aa, hpost[:, o:o + w],
                                 start=True, stop=False)
                nc.tensor.matmul(pp[:, o:o + w], wfc_ba, eT[:, o:o + w],
                                 start=False, stop=True)
            for (o, w) in CHUNKS:
                nc.tensor.matmul(gqp[:64, o:o + w], wfc_ab,
                                 hpost[:, o:o + w], start=True, stop=False)
                nc.tensor.matmul(gqp[:64, o:o + w], wfc_bb, eT[:, o:o + w],
                                 start=False, stop=True)
            elu_chunk(hfca, pp, small["sc_fc_a"], small["sh_fc_a"],
                      pe1, pe2)
            elu_chunk(hfcb, gqp[:64, :PC], small["sc_fc_b"],
                      small["sh_fc_b"], gtc[:64, :], gfc[:64, :])
            # logits (feature-major); |logits| < 3 so exp is safe
            for (o, w) in CHUNKS:
                nc.tensor.matmul(pp[:NOUT, o:o + w], w_out_a,
                                 hfca[:, o:o + w], start=True, stop=False)
                nc.tensor.matmul(pp[:NOUT, o:o + w], w_out_b,
                                 hfcb[:, o:o + w], start=False, stop=True)
            nc.vector.tensor_scalar(ty, pp[:NOUT, :], bout_col, None,
                                    OP.add)
            nc.scalar.activation(tex, pp[:NOUT, :], AF.Exp, bias=bout_col)
            for ci, (o, w) in enumerate(CHUNKS):
                nc.tensor.matmul(pp[:1, o:o + w], ones20_bf,
                                 tex[:, o:o + w], start=True, stop=True)
            nc.scalar.activation(t3a[:, :PC], pp[:1, :PC], AF.Ln)
            for ci, (o, w) in enumerate(CHUNKS):
                nc.tensor.matmul(pp[:NOUT, o:o + w], ones_r20,
                                 t3a[:1, o:o + w], start=True, stop=True)
            nc.vector.tensor_tensor(out_sb, ty, pp[:NOUT, :PC],
                                    OP.subtract)
            for it in range(IT):
                nc.tensor.transpose(
                    pp[:, it * NOUT:(it + 1) * NOUT],
                    out_sb[:, it * 128:(it + 1) * 128], ident20)
            nc.vector.tensor_copy(outall, pp[:, :IT * NOUT])
            nc.sync.dma_start(d_out.ap(), outall)

    nc.compile()
    return nc


# --------------------------------------------------------------------------
# host side
# --------------------------------------------------------------------------

def _stage_inputs(
    x, embed, adj, W_in, b_in, bn_in_g, bn_in_b, bn_in_rm, bn_in_rv,
    W_gs, b_gs, Wih0, Whh0, bih0, bhh0, Wih1, Whh1, bih1, bhh1,
    W_emb, b_emb, bn_emb_g, bn_emb_b, bn_emb_rm, bn_emb_rv,
    W_fc, b_fc, bn_fc_g, bn_fc_b, bn_fc_rm, bn_fc_rv, W_out, b_out,
):
    x = np.asarray(x, np.float32)
    embed = np.asarray(embed, np.float32)
    adj = np.asarray(adj, np.float32)

    w_inT = np.zeros((FPAD, NH), ml_dtypes.bfloat16)
    w_inT[:NFEAT] = _bf(np.asarray(W_in, np.float32).T)

    def bn_fold(g, b, rm, rv, lin_b=None):
        g = np.asarray(g, np.float32); b = np.asarray(b, np.float32)
        rm = np.asarray(rm, np.float32); rv = np.asarray(rv, np.float32)
        sc = g / np.sqrt(rv + BN_EPS)
        base = lin_b if lin_b is not None else 0.0
        shv = sc * (base - rm) + b
        return _f32(sc), _f32(shv)

    sc_in, sh_in = bn_fold(bn_in_g, bn_in_b, bn_in_rm, bn_in_rv,
                           np.asarray(b_in, np.float32))
    _, sh_in2 = bn_fold(bn_in_g, bn_in_b, bn_in_rm, bn_in_rv)
    sc_emb, sh_emb = bn_fold(bn_emb_g, bn_emb_b, bn_emb_rm, bn_emb_rv,
                             np.asarray(b_emb, np.float32))
    sc_fc, sh_fc = bn_fold(bn_fc_g, bn_fc_b, bn_fc_rm, bn_fc_rv,
                           np.asarray(b_fc, np.float32))

    W_gs = np.asarray(W_gs, np.float32)
    wgs_sT = _bf(np.stack([W_gs[l][:, :NH].T for l in range(L)]))
    wgs_nT = _bf(np.stack([W_gs[l][:, NH:].T for l in range(L)])
                 * (1.0 / ADJ_SCALE))
    bgs = _f32(np.asarray(b_gs, np.float32).T)          # [NH, L]

    # tanh-only LSTM: layer-1 inputs and all h_prev are DOUBLED h states,
    # so Wih1 and both Whh get 0.5 folded in; additionally the sigmoid
    # gates (i/f/o) are computed as tanh(z/2+b/2), so those gate blocks of
    # W and b get another 0.5 -- activations then all run at scale=1.
    wih_l = [np.asarray(Wih0, np.float32).T.copy(),
             0.5 * np.asarray(Wih1, np.float32).T]
    whh_l = [0.5 * np.asarray(Whh0, np.float32).T,
             0.5 * np.asarray(Whh1, np.float32).T]
    for arr in wih_l + whh_l:
        for g in (0, 1, 3):
            arr[:, g * NH:(g + 1) * NH] *= 0.5
    wihT = np.stack([_bf(wih_l[0]), _bf(wih_l[1])])
    whhT = np.stack([_bf(whh_l[0]), _bf(whh_l[1])])
    bl = np.stack([np.asarray(bih0, np.float32) + np.asarray(bhh0, np.float32),
                   np.asarray(bih1, np.float32) + np.asarray(bhh1, np.float32)])
    blstm = np.zeros((NH, 8), np.float32)
    for l in range(2):
        for g in range(4):
            f = 1.0 if g == 2 else 0.5
            blstm[:, l * 4 + g] = f * bl[l][g * NH:(g + 1) * NH]
    # same biases as a row vector for the rank-1 bias matmuls (tail cells)
    blr = np.zeros((1, 8 * NH), ml_dtypes.bfloat16)
    for l in range(2):
        for g in range(4):
            blr[0, (l * 4 + g) * NH:(l * 4 + g + 1) * NH] = \
                _bf(blstm[:, l * 4 + g])

    shared = {
        "w_inT": w_inT,
        "wgs_sT": wgs_sT, "wgs_nT": wgs_nT, "bgs": bgs,
        "wihT": _bf(wihT), "whhT": _bf(whhT), "blstm": blstm, "blr": blr,
        "w_embT": _bf(np.asarray(W_emb, np.float32).T),
        "w_fcT": _bf(np.asarray(W_fc, np.float32).T),
        "w_outT": _bf(np.asarray(W_out, np.float32).T),
        "bout_col": _f32(np.asarray(b_out, np.float32))[:, None],
        "sc_in": sc_in[:, None], "sh_in": sh_in[:, None],
        # JK mean of two DOUBLED h states: 0.5 * 0.5 = 0.25
        "sc_in_h": _f32(0.25 * sc_in)[:, None], "sh_in2": sh_in2[:, None],
        "sc_emb": sc_emb[:, None], "sh_emb": sh_emb[:, None],
        "sc_fc_a": _f32(sc_fc[:128])[:, None], "sh_fc_a": _f32(sh_fc[:128])[:, None],
        "sc_fc_b": _f32(sc_fc[128:])[:, None], "sh_fc_b": _f32(sh_fc[128:])[:, None],
    }

    rowsum = adj.sum(axis=1)                    # fp32, exact rows
    in_maps = []
    for c in range(NC):
        rows = slice(c * NPC, (c + 1) * NPC)
        scaled = adj[rows] * (ADJ_SCALE / rowsum[rows])[:, None]
        at = scaled.T                           # [10000, 1250]
        padded = np.zeros((NP, PC), np.float32)
        for ck in range(NC):
            padded[ck * PC:ck * PC + NPC, :NPC] = at[ck * NPC:(ck + 1) * NPC]
        adj8 = padded.astype(ml_dtypes.float8_e4m3fn)
        # [t, p, i] -> [h, g, p, s, i] with t = g*10 + h*5 + s
        adj8 = np.ascontiguousarray(
            adj8.reshape(NC, 2, HT, 128, PC).transpose(1, 0, 3, 2, 4))

        xT = np.zeros((FPAD, PC), ml_dtypes.bfloat16)
        xT[:NFEAT, :NPC] = _bf(x[rows].T)
        x8 = np.ascontiguousarray(
            xT.reshape(FT, 128, NE, EW).transpose(2, 1, 0, 3))

        embT = np.zeros((NFE, PC), ml_dtypes.bfloat16)
        embT[:, :NPC] = _bf(embed[rows].T)
        embT = np.ascontiguousarray(embT.reshape(2, 128, PC).transpose(1, 0, 2))

        m = {"adj8": adj8, "x8": x8, "embT": embT}
        m.update(shared)
        in_maps.append(m)
    return in_maps


def kernel(**inputs) -> np.ndarray:
    global _CACHED_NC, LAST_RESULT
    in_maps = _stage_inputs(**inputs)
    if _CACHED_NC is None:
        _CACHED_NC = _build_program()
    nc = _CACHED_NC
    trace = bool(int(os.environ.get("GSAGE_TRACE", "0")))
    res = run_bass_kernel_spmd(
        nc, in_maps, core_ids=list(range(NC)), trace=trace,
    )
    LAST_RESULT = res
    parts = []
    for c in range(NC):
        o = np.asarray(res.results[c]["out"], np.float32)
        o = o.reshape(128, IT, NOUT).transpose(1, 0, 2).reshape(PC, NOUT)
        parts.append(o[:NPC])
    out = np.concatenate(parts, axis=0)
    return np.ascontiguousarray(out, np.float32)


if __name__ == "__main__":
    import reference
    inputs = reference.setup_inputs()
    out = kernel(**{k: np.asarray(v) for k, v in inputs.items()})
    print("out", out.shape, out.dtype)
